# revision 44
# baseline (speedup 1.0000x reference)
"""Contrastive loss (NT-Xent) on 8 Trainium2 cores — v3d.

Symmetric 5-panel scheme: sim = z z^T is symmetric; each core computes its
1024-row block against 5 of 8 column panels (panels c..c+4 in rotated
coordinates).  Denominator coverage:
  panel 0 (diagonal block): symmetric, so its row sums equal its column
    sums -> computed as PARTITION-sums on the tensor engine (ones-matmul
    over exp'd bf16 tiles, accumulated across row tiles).
  panels 1..3: row sums on the exp engines (ACT accum_out / DVE reduce);
    COLUMN sums (tensor engine) are shipped to cores c+5..c+7, whose rows
    they cover by symmetry.
  panel 4 (antipodal block): cores come in pairs (c, c+4) that compute
    transposed blocks of each other -> each ships its PARTITION-sums
    (column sums) to the partner instead of reducing its own rows.
Final assembly (denominator gather, diag correction, log, scalar loss)
runs on the host in f64 — the all-reduce/unshard step.

Host-side input prep (the shard/layout step): x^T in fp8, rotated by
-c*1024 columns per core, first 5120 columns, plus the reciprocal-norm
row r = 16/||x8_j|| in bf16.

Device per column group g: rbc broadcast DMA (ACT DMA queue) -> z8 =
fp8(x8 * rbc) on gpsimd -> sim row-tiles [128,1024] fp8 DoubleRow GEMM ->
exp split: ACT tiles (activation Exp, bf16 out) / DVE tiles (Schraudolph
int16 affine, bf16 bitcast); rowsum accum only for panels 1..3; panels
0/4: DVE pair-adds then ones-matmul partition sums; colsum ones-matmuls
for panels 1..3 -> raw partials DMA'd out.
"""

import numpy as np
import ml_dtypes

import concourse.bass as bass
import concourse.tile as tile
from concourse import bacc, mybir

F32 = mybir.dt.float32
BF16 = mybir.dt.bfloat16
FP8 = mybir.dt.float8e4
I16 = mybir.dt.int16
AF = mybir.ActivationFunctionType
ALU = mybir.AluOpType
AX = mybir.AxisListType
PSUM = bass.MemorySpace.PSUM
DR = mybir.MatmulPerfMode.DoubleRow

N = 4096
TWO_N = 2 * N
D = 256
RPC = TWO_N // 8          # rows per core = 1024
M_TILES = RPC // 128      # 8 row tiles per core
G = 5                     # column panels per core (symmetric scheme)
SW = G * RPC              # streamed columns = 5120

TAU_INV = 10.0
SCALE_Z = 16.0
K_SIM = TAU_INV / (SCALE_Z * SCALE_Z)
LOG2E = 1.4426950408889634
S16 = float(np.float32((2.0**7) * LOG2E * K_SIM))
B16 = float(np.float32((2.0**7) * 127 - 470000.0 / (2.0**16)))

# per-g engine split of the 8 row tiles: A = scalar-act exp, D = DVE
# Schraudolph.
TILE_ENGINE = "AADADADA"


def build_nc(nc=None):
    if nc is None:
        nc = bacc.Bacc("TRN2", target_bir_lowering=False, debug=False)

    xt_d = nc.declare_dram_parameter("xt", [128, 2 * SW], FP8, isOutput=False)
    r_d = nc.declare_dram_parameter("r", [128, SW], BF16, isOutput=False)
    den_d = nc.declare_dram_parameter("den", [128, M_TILES * G], F32,
                                      isOutput=True)
    # single row output: colsums g1..3 | pos | self | g0 triangle colsums
    orow_d = nc.declare_dram_parameter("orow", [1, 6 * RPC], F32,
                                       isOutput=True)

    with tile.TileContext(nc) as tc:
        with (
            tc.tile_pool(name="const", bufs=1) as cpool,
            tc.tile_pool(name="xt", bufs=1) as xt_pool,
            tc.tile_pool(name="zt", bufs=1) as zt_pool,
            tc.tile_pool(name="rbc", bufs=3) as rbc_pool,
            tc.tile_pool(name="expd", bufs=4) as expd_pool,
            tc.tile_pool(name="ti", bufs=4) as ti_pool,
            tc.tile_pool(name="fin", bufs=1) as fin_pool,
        ):
            ones_bf = cpool.tile([128, 1], BF16, name="ones_bf", tag="ones_bf")
            nc.vector.memset(ones_bf[:], 1.0)

            # PE warm-up: the HAM clock gate halves the PE clock after
            # ~3.4us of idle, and the DMA lead-in guarantees that.  A burst
            # of dummy matmuls during the input DMAs un-throttles the PE
            # before the first real GEMM arrives.
            with tc.tile_pool(name="warm", bufs=1, space=PSUM) as wpool:
                wsrc = cpool.tile([128, 512], BF16, name="wsrc", tag="wsrc")
                nc.vector.memset(wsrc[:], 1.0)
                wp = wpool.tile([1, 512], F32, name="wp", tag="wp")
                # long enough to bridge until z8(0) is ready (~17us), else
                # the HAM re-throttles during the gap and the warm-up is lost
                for _ in range(26):
                    nc.tensor.matmul(wp[:], ones_bf[:], wsrc[:],
                                     start=True, stop=True)

            xt8 = xt_pool.tile([128, 2, SW], FP8, name="xt8", tag="xt8")
            zt8 = zt_pool.tile([128, 2, SW], FP8, name="zt8", tag="zt8")

            den_acc = fin_pool.tile([128, M_TILES * G], F32, name="den_acc",
                                    tag="den_acc")

            # pre-broadcast reciprocal norms shipped from host [128, SW]
            rbc8 = rbc_pool.tile([128, SW], BF16, name="rbc8", tag="rbc8")

            # ---- input DMAs: group 0 first on the sync queue; the rest
            # split across the sync and ACT hardware DMA queues so the
            # ~600ns/descriptor serialization overlaps
            xt_v = xt_d[:].rearrange("p (k j) -> p k j", k=2)
            g0 = slice(0, RPC)
            rest = slice(RPC, SW)
            nc.sync.dma_start(xt8[:, :, g0], xt_v[:, :, g0])
            nc.sync.dma_start(rbc8[:, g0], r_d[:, g0])
            nc.sync.dma_start(xt8[:, :, rest], xt_v[:, :, rest])
            nc.scalar.dma_start(rbc8[:, rest], r_d[:, rest])

            def prep_z8(g, eng):
                gs = slice(g * RPC, (g + 1) * RPC)
                eng.tensor_mul(zt8[:, 0, gs], xt8[:, 0, gs], rbc8[:, gs])
                eng.tensor_mul(zt8[:, 1, gs], xt8[:, 1, gs], rbc8[:, gs])

            # ================= main loop =================
            def do_tile(g, m, sim_pool):
                """GEMM + exp for tile (g, m); returns the exp'd bf16 view.

                g0 is the symmetric diagonal block: only columns
                [m*128, 1024) are computed (diag chunk + strict upper);
                the below-diagonal part is recovered from column sums of
                the strict-upper region by symmetry.
                """
                ms = slice(m * 128, (m + 1) * 128)
                w = RPC
                lo = g * RPC + (RPC - w)
                st = sim_pool.tile([128, RPC], F32, name="sim", tag="sim")
                for j0 in range(0, w, 512):
                    jw = min(512, w - j0)
                    nc.tensor.matmul(st[:, j0:j0 + jw],
                                     zt8[:, :, ms],
                                     zt8[:, :, lo + j0:lo + j0 + jw],
                                     start=True, stop=True, perf_mode=DR)
                dcol = den_acc[:, m * G + g:m * G + g + 1]
                if TILE_ENGINE[m] == "A":
                    if g in (0, 4):
                        # nobody reads the exp'd tile: write PSUM in place
                        nc.scalar.activation(st[:], st[:], AF.Exp,
                                             scale=K_SIM, accum_out=dcol)
                        return None
                    eb = expd_pool.tile([128, RPC], BF16, name="eb", tag="eb")
                    nc.scalar.activation(eb[:, 0:w], st[:, 0:w], AF.Exp,
                                         scale=K_SIM, accum_out=dcol)
                    return eb
                else:
                    ti = ti_pool.tile([128, RPC], I16, name="ti", tag="ti")
                    nc.vector.tensor_scalar(ti[:, 0:w], st[:, 0:w], S16, B16,
                                            ALU.mult, ALU.add)
                    nc.vector.tensor_reduce(dcol,
                                            ti[:, 0:w].bitcast(BF16),
                                            axis=AX.X, op=ALU.add)
                    return ti.bitcast(BF16)

            prod_a = [
                fin_pool.tile([128, RPC], BF16, name=f"prod_a{k}",
                              tag=f"prod_a{k}")
                for k in range(2)
            ]
            prod_s = [
                fin_pool.tile([128, RPC], BF16, name=f"prod_s{k}",
                              tag=f"prod_s{k}")
                for k in range(2)
            ]

            with (
                tc.tile_pool(name="simp", bufs=3, space=PSUM) as sim_pool,
                tc.tile_pool(name="csp", bufs=1, space=PSUM) as cs_pool,
            ):
                # group 0 split DVE/gpsimd (both idle at start, runs in
                # parallel) so the first GEMM can begin early; the rest
                # stream on gpsimd
                gs0 = slice(0, RPC)
                nc.vector.tensor_mul(zt8[:, 0, gs0], xt8[:, 0, gs0],
                                     rbc8[:, gs0])
                nc.gpsimd.tensor_mul(zt8[:, 1, gs0], xt8[:, 1, gs0],
                                     rbc8[:, gs0])
                for g2 in range(1, G):
                    prep_z8(g2, nc.gpsimd)

                # single consolidated output row in SBUF
                orow = fin_pool.tile([1, 6 * RPC], F32, name="orow",
                                     tag="orow")

                # deferred evacuations: list of (psum_half, orow_offset, eng)
                pending = []

                def flush_pending():
                    for ps, off, eng in pending:
                        if eng == "A":
                            nc.scalar.copy(orow[0:1, off:off + 512], ps[:])
                        else:
                            nc.vector.tensor_copy(orow[0:1, off:off + 512],
                                                  ps[:])
                    pending.clear()

                def cs_mms(g, mm, csh, expd, last):
                    """colsum matmuls for tile (g, mm).  g0: only the
                    strict-upper region, with tile-local offset mapping."""
                    for h in range(2):
                        lo_p = h * 512
                        hi_p = lo_p + 512
                        if g == 0:
                            lo_p = max(lo_p, (mm + 1) * 128)
                        if lo_p >= hi_p:
                            continue
                        ofs = mm * 128 if g == 0 else 0
                        nc.tensor.matmul(
                            csh[h][0:1, lo_p - h * 512:hi_p - h * 512],
                            ones_bf[:], expd[:, lo_p - ofs:hi_p - ofs],
                            start=(mm == 0), stop=last)

                for g in range(G):
                    colsum = 1 <= g <= 3
                    csh = None
                    if colsum:
                        csh = [cs_pool.tile([1, 512], F32, name=f"csh{h}",
                                            tag=f"csh{h}") for h in range(2)]
                    expd_tiles = {}
                    for m in range(M_TILES):
                        eb = do_tile(g, m, sim_pool)
                        expd_tiles[m] = eb
                        # column sums TWO tiles behind, batched in pairs:
                        # the PE never waits on the exp engines, and the
                        # ones<->zt8 stationary swaps halve
                        if colsum and m in (3, 5, 7):
                            for mm in (m - 3, m - 2):
                                cs_mms(g, mm, csh, expd_tiles[mm], False)
                        if m == 0:
                            flush_pending()
                        if m == 1 and g == 0:
                            nc.gpsimd.tensor_mul(prod_s[1][:],
                                                 zt8[:, 1, 0:RPC],
                                                 zt8[:, 1, 0:RPC])
                        if m == 1 and g == 1:
                            nc.gpsimd.tensor_mul(prod_s[0][:],
                                                 zt8[:, 0, 0:RPC],
                                                 zt8[:, 0, 0:RPC])
                        if m == 1 and g == 2:
                            nc.gpsimd.tensor_mul(prod_a[1][:],
                                                 zt8[:, 1, 0:RPC],
                                                 zt8[:, 1, 4 * RPC:5 * RPC])
                        if m == 3 and g == 2:
                            nc.gpsimd.tensor_mul(prod_a[0][:],
                                                 zt8[:, 0, 0:RPC],
                                                 zt8[:, 0, 4 * RPC:5 * RPC])
                        # pos/self diagonal sums ride the colsum PSUM banks
                        # freed during g4 (csh tags, ring reuse)
                        if g == 4 and m in (2, 5):
                            prod = prod_s if m == 2 else prod_a
                            off = (4 if m == 2 else 3) * RPC
                            ph = [cs_pool.tile([1, 512], F32, name=f"ph{h}",
                                               tag=f"csh{h}")
                                  for h in range(2)]
                            for h in range(2):
                                js = slice(h * 512, (h + 1) * 512)
                                for k in range(2):
                                    nc.tensor.matmul(
                                        ph[h][0:1, :], ones_bf[:],
                                        prod[k][:, js],
                                        start=(k == 0), stop=(k == 1))
                            pending.append((ph[0], off, "A"))
                            pending.append((ph[1], off + 512, "D"))
                        if g == 4 and m in (4, 7):
                            flush_pending()
                    if colsum:
                        lastm = M_TILES - 1 if g != 0 else M_TILES - 2
                        for mm in (M_TILES - 2, M_TILES - 1):
                            cs_mms(g, mm, csh, expd_tiles[mm], mm == lastm)
                        base = 5 * RPC if g == 0 else (g - 1) * RPC
                        pending.append((csh[0], base, "A"))
                        pending.append((csh[1], base + 512, "D"))
                flush_pending()
                nc.sync.dma_start(den_d[:], den_acc[:])
                nc.sync.dma_start(orow_d[:], orow[:])

    nc.compile()
    return nc


_NC = None


def _get_nc():
    global _NC
    if _NC is None:
        _NC = build_nc()
    return _NC


def make_in_maps(x1, x2):
    x1 = np.asarray(x1, dtype=np.float32)
    x2 = np.asarray(x2, dtype=np.float32)
    x = np.concatenate([x1, x2], axis=0)               # [8192, 256]
    xT8 = np.ascontiguousarray(x.T).astype(ml_dtypes.float8_e4m3fn)
    # reciprocal norms of the fp8-quantized columns, bf16 (device semantics)
    ssq = (xT8.astype(np.float32) ** 2).sum(axis=0)    # [8192]
    r_full = (SCALE_Z / np.sqrt(ssq)).astype(ml_dtypes.bfloat16)
    in_maps = []
    for c in range(8):
        xr = np.roll(xT8, -c * RPC, axis=1)[:, :SW]
        arr = np.stack([xr[:128], xr[128:]], axis=1)   # [128, 2, SW]
        rr = np.roll(r_full, -c * RPC)[:SW]
        in_maps.append({
            "xt": np.ascontiguousarray(arr.reshape(128, 2 * SW)),
            "r": np.ascontiguousarray(
                np.broadcast_to(rr[None, :], (128, SW))),
        })
    return in_maps


def _combine(results):
    """Host-side unshard: assemble denominators, diag correction, loss."""
    den_total = np.zeros(TWO_N, dtype=np.float64)
    pos_sum = 0.0
    for c in range(8):
        r = results[c]
        den_own = np.asarray(r["den"], dtype=np.float64)   # [128, 40]
        orow = np.asarray(r["orow"], dtype=np.float64).reshape(6 * RPC)
        # rowsums over all 5 panels (g0 covers cols >= its diag chunk);
        # local row i = m*128 + p
        den_rows = den_own.reshape(128, M_TILES, G).sum(axis=2)
        den_rows = den_rows.T.reshape(RPC)                  # [i]
        slf = orow[4 * RPC:5 * RPC]
        den_rows = den_rows + 1.0 - np.exp(K_SIM * slf)
        lo = c * RPC
        den_total[lo:lo + RPC] += den_rows
        cs = orow[:3 * RPC].reshape(3, RPC)
        for g in (1, 2, 3):
            dest = ((c + g) * RPC) % TWO_N
            den_total[dest:dest + RPC] += cs[g - 1]
        pos_sum += float(orow[3 * RPC:4 * RPC].sum())
    loss = (np.log(den_total).sum() - K_SIM * pos_sum) / TWO_N
    return np.asarray(np.float32(loss))


def _run(x1, x2, trace=False, tmpdir=None):
    from concourse.bass_utils import run_bass_kernel_spmd

    nc = _get_nc()
    in_maps = make_in_maps(x1, x2)
    res = run_bass_kernel_spmd(
        nc, in_maps, list(range(8)), trace=trace, tmpdir=tmpdir
    )
    loss = _combine(res.results)
    return loss, res


def kernel(x1, x2):
    loss, _ = _run(x1, x2)
    return loss


# revision 47
# speedup vs baseline: 1.1893x; 1.1893x over previous
"""Contrastive loss (NT-Xent) on 8 Trainium2 cores — v3d.

Symmetric 5-panel scheme: sim = z z^T is symmetric; each core computes its
1024-row block against 5 of 8 column panels (panels c..c+4 in rotated
coordinates).  Denominator coverage:
  panel 0 (diagonal block): symmetric, so its row sums equal its column
    sums -> computed as PARTITION-sums on the tensor engine (ones-matmul
    over exp'd bf16 tiles, accumulated across row tiles).
  panels 1..3: row sums on the exp engines (ACT accum_out / DVE reduce);
    COLUMN sums (tensor engine) are shipped to cores c+5..c+7, whose rows
    they cover by symmetry.
  panel 4 (antipodal block): cores come in pairs (c, c+4) that compute
    transposed blocks of each other -> each ships its PARTITION-sums
    (column sums) to the partner instead of reducing its own rows.
Final assembly (denominator gather, diag correction, log, scalar loss)
runs on the host in f64 — the all-reduce/unshard step.

Host-side input prep (the shard/layout step): x^T in fp8, rotated by
-c*1024 columns per core, first 5120 columns, plus the reciprocal-norm
row r = 16/||x8_j|| in bf16.

Device per column group g: rbc broadcast DMA (ACT DMA queue) -> z8 =
fp8(x8 * rbc) on gpsimd -> sim row-tiles [128,1024] fp8 DoubleRow GEMM ->
exp split: ACT tiles (activation Exp, bf16 out) / DVE tiles (Schraudolph
int16 affine, bf16 bitcast); rowsum accum only for panels 1..3; panels
0/4: DVE pair-adds then ones-matmul partition sums; colsum ones-matmuls
for panels 1..3 -> raw partials DMA'd out.
"""

import numpy as np
import ml_dtypes

import concourse.bass as bass
import concourse.tile as tile
from concourse import bacc, mybir

F32 = mybir.dt.float32
BF16 = mybir.dt.bfloat16
FP8 = mybir.dt.float8e4
I16 = mybir.dt.int16
AF = mybir.ActivationFunctionType
ALU = mybir.AluOpType
AX = mybir.AxisListType
PSUM = bass.MemorySpace.PSUM
DR = mybir.MatmulPerfMode.DoubleRow

N = 4096
TWO_N = 2 * N
D = 256
RPC = TWO_N // 8          # rows per core = 1024
M_TILES = RPC // 128      # 8 row tiles per core
G = 5                     # column panels per core (symmetric scheme)
SW = G * RPC              # streamed columns = 5120

TAU_INV = 10.0
SCALE_Z = 16.0
K_SIM = TAU_INV / (SCALE_Z * SCALE_Z)
LOG2E = 1.4426950408889634
S16 = float(np.float32((2.0**7) * LOG2E * K_SIM))
B16 = float(np.float32((2.0**7) * 127 - 470000.0 / (2.0**16)))

# per-g engine split of the 8 row tiles: A = scalar-act exp, D = DVE
# Schraudolph.  The last group ends with two A-tiles so the pipeline
# drain after the final GEMM is the short ACT chain, not a DVE TS+TR.
TILE_ENGINES = ["AADADADA"] * 4 + ["ADADADAA"]


def build_nc(nc=None):
    if nc is None:
        nc = bacc.Bacc("TRN2", target_bir_lowering=False, debug=False)

    xt_d = nc.declare_dram_parameter("xt", [128, 2 * SW], FP8, isOutput=False)
    r_d = nc.declare_dram_parameter("r", [128, SW], BF16, isOutput=False)
    den_d = nc.declare_dram_parameter("den", [128, M_TILES * G], F32,
                                      isOutput=True)
    # single row output: colsums g1..3 | pos | self | g0 triangle colsums
    orow_d = nc.declare_dram_parameter("orow", [1, 6 * RPC], F32,
                                       isOutput=True)

    with tile.TileContext(nc) as tc:
        with (
            tc.tile_pool(name="const", bufs=1) as cpool,
            tc.tile_pool(name="xt", bufs=1) as xt_pool,
            tc.tile_pool(name="zt", bufs=1) as zt_pool,
            tc.tile_pool(name="rbc", bufs=3) as rbc_pool,
            tc.tile_pool(name="expd", bufs=4) as expd_pool,
            tc.tile_pool(name="ti", bufs=4) as ti_pool,
            tc.tile_pool(name="fin", bufs=1) as fin_pool,
        ):
            ones_bf = cpool.tile([128, 1], BF16, name="ones_bf", tag="ones_bf")
            nc.vector.memset(ones_bf[:], 1.0)

            xt8 = xt_pool.tile([128, 2, SW], FP8, name="xt8", tag="xt8")
            zt8 = zt_pool.tile([128, 2, SW], FP8, name="zt8", tag="zt8")

            den_acc = fin_pool.tile([128, M_TILES * G], F32, name="den_acc",
                                    tag="den_acc")

            # pre-broadcast reciprocal norms shipped from host [128, SW]
            rbc8 = rbc_pool.tile([128, SW], BF16, name="rbc8", tag="rbc8")

            # ---- input DMAs: group 0 first on the sync queue; the rest
            # split across the sync and ACT hardware DMA queues so the
            # ~600ns/descriptor serialization overlaps
            xt_v = xt_d[:].rearrange("p (k j) -> p k j", k=2)
            g0 = slice(0, RPC)
            rest = slice(RPC, SW)
            nc.sync.dma_start(xt8[:, :, g0], xt_v[:, :, g0])
            nc.sync.dma_start(rbc8[:, g0], r_d[:, g0])
            nc.sync.dma_start(xt8[:, :, rest], xt_v[:, :, rest])
            nc.scalar.dma_start(rbc8[:, rest], r_d[:, rest])

            def prep_z8(g, eng):
                gs = slice(g * RPC, (g + 1) * RPC)
                eng.tensor_mul(zt8[:, 0, gs], xt8[:, 0, gs], rbc8[:, gs])
                eng.tensor_mul(zt8[:, 1, gs], xt8[:, 1, gs], rbc8[:, gs])

            # ================= main loop =================
            def do_tile(g, m, sim_pool):
                """GEMM + exp for tile (g, m); returns the exp'd bf16 view.

                g0 is the symmetric diagonal block: only columns
                [m*128, 1024) are computed (diag chunk + strict upper);
                the below-diagonal part is recovered from column sums of
                the strict-upper region by symmetry.
                """
                ms = slice(m * 128, (m + 1) * 128)
                w = RPC
                lo = g * RPC + (RPC - w)
                st = sim_pool.tile([128, RPC], F32, name="sim", tag="sim")
                for j0 in range(0, w, 512):
                    jw = min(512, w - j0)
                    nc.tensor.matmul(st[:, j0:j0 + jw],
                                     zt8[:, :, ms],
                                     zt8[:, :, lo + j0:lo + j0 + jw],
                                     start=True, stop=True, perf_mode=DR)
                dcol = den_acc[:, m * G + g:m * G + g + 1]
                if TILE_ENGINES[g][m] == "A":
                    if g in (0, 4):
                        # nobody reads the exp'd tile: write PSUM in place
                        nc.scalar.activation(st[:], st[:], AF.Exp,
                                             scale=K_SIM, accum_out=dcol)
                        return None
                    eb = expd_pool.tile([128, RPC], BF16, name="eb", tag="eb")
                    nc.scalar.activation(eb[:, 0:w], st[:, 0:w], AF.Exp,
                                         scale=K_SIM, accum_out=dcol)
                    return eb
                else:
                    ti = ti_pool.tile([128, RPC], I16, name="ti", tag="ti")
                    nc.vector.tensor_scalar(ti[:, 0:w], st[:, 0:w], S16, B16,
                                            ALU.mult, ALU.add)
                    nc.vector.tensor_reduce(dcol,
                                            ti[:, 0:w].bitcast(BF16),
                                            axis=AX.X, op=ALU.add)
                    return ti.bitcast(BF16)

            prod_a = [
                fin_pool.tile([128, RPC], BF16, name=f"prod_a{k}",
                              tag=f"prod_a{k}")
                for k in range(2)
            ]
            prod_s = [
                fin_pool.tile([128, RPC], BF16, name=f"prod_s{k}",
                              tag=f"prod_s{k}")
                for k in range(2)
            ]

            with (
                tc.tile_pool(name="simp", bufs=3, space=PSUM) as sim_pool,
                tc.tile_pool(name="csp", bufs=1, space=PSUM) as cs_pool,
            ):
                # group 0 split DVE/gpsimd (both idle at start, runs in
                # parallel) so the first GEMM can begin early; the rest
                # stream on gpsimd
                gs0 = slice(0, RPC)
                nc.vector.tensor_mul(zt8[:, 0, gs0], xt8[:, 0, gs0],
                                     rbc8[:, gs0])
                nc.gpsimd.tensor_mul(zt8[:, 1, gs0], xt8[:, 1, gs0],
                                     rbc8[:, gs0])
                for g2 in range(1, G):
                    prep_z8(g2, nc.gpsimd)

                # single consolidated output row in SBUF
                orow = fin_pool.tile([1, 6 * RPC], F32, name="orow",
                                     tag="orow")

                # deferred evacuations: list of (psum_half, orow_offset, eng)
                pending = []

                def flush_pending():
                    for ps, off, eng in pending:
                        if eng == "A":
                            nc.scalar.copy(orow[0:1, off:off + 512], ps[:])
                        else:
                            nc.vector.tensor_copy(orow[0:1, off:off + 512],
                                                  ps[:])
                    pending.clear()

                def cs_mms(g, mm, csh, expd, last):
                    """colsum matmuls for tile (g, mm).  g0: only the
                    strict-upper region, with tile-local offset mapping."""
                    for h in range(2):
                        lo_p = h * 512
                        hi_p = lo_p + 512
                        if g == 0:
                            lo_p = max(lo_p, (mm + 1) * 128)
                        if lo_p >= hi_p:
                            continue
                        ofs = mm * 128 if g == 0 else 0
                        nc.tensor.matmul(
                            csh[h][0:1, lo_p - h * 512:hi_p - h * 512],
                            ones_bf[:], expd[:, lo_p - ofs:hi_p - ofs],
                            start=(mm == 0), stop=last)

                for g in range(G):
                    colsum = 1 <= g <= 3
                    csh = None
                    if colsum:
                        csh = [cs_pool.tile([1, 512], F32, name=f"csh{h}",
                                            tag=f"csh{h}") for h in range(2)]
                    expd_tiles = {}
                    for m in range(M_TILES):
                        eb = do_tile(g, m, sim_pool)
                        expd_tiles[m] = eb
                        # column sums TWO tiles behind, batched in pairs:
                        # the PE never waits on the exp engines, and the
                        # ones<->zt8 stationary swaps halve
                        if colsum and m in (3, 5, 7):
                            for mm in (m - 3, m - 2):
                                cs_mms(g, mm, csh, expd_tiles[mm], False)
                        if m == 0:
                            flush_pending()
                        if m == 1 and g == 0:
                            nc.gpsimd.tensor_mul(prod_s[1][:],
                                                 zt8[:, 1, 0:RPC],
                                                 zt8[:, 1, 0:RPC])
                        if m == 1 and g == 1:
                            nc.gpsimd.tensor_mul(prod_s[0][:],
                                                 zt8[:, 0, 0:RPC],
                                                 zt8[:, 0, 0:RPC])
                        if m == 1 and g == 2:
                            nc.gpsimd.tensor_mul(prod_a[1][:],
                                                 zt8[:, 1, 0:RPC],
                                                 zt8[:, 1, 4 * RPC:5 * RPC])
                        if m == 3 and g == 2:
                            nc.gpsimd.tensor_mul(prod_a[0][:],
                                                 zt8[:, 0, 0:RPC],
                                                 zt8[:, 0, 4 * RPC:5 * RPC])
                        # pos/self diagonal sums ride the colsum PSUM banks
                        # freed during g4 (csh tags, ring reuse)
                        if g == 4 and m in (2, 5):
                            prod = prod_s if m == 2 else prod_a
                            off = (4 if m == 2 else 3) * RPC
                            ph = [cs_pool.tile([1, 512], F32, name=f"ph{h}",
                                               tag=f"csh{h}")
                                  for h in range(2)]
                            for h in range(2):
                                js = slice(h * 512, (h + 1) * 512)
                                for k in range(2):
                                    nc.tensor.matmul(
                                        ph[h][0:1, :], ones_bf[:],
                                        prod[k][:, js],
                                        start=(k == 0), stop=(k == 1))
                            pending.append((ph[0], off, "A"))
                            pending.append((ph[1], off + 512, "D"))
                        if g == 4 and m in (4, 7):
                            flush_pending()
                    if colsum:
                        lastm = M_TILES - 1 if g != 0 else M_TILES - 2
                        for mm in (M_TILES - 2, M_TILES - 1):
                            cs_mms(g, mm, csh, expd_tiles[mm], mm == lastm)
                        base = 5 * RPC if g == 0 else (g - 1) * RPC
                        pending.append((csh[0], base, "A"))
                        pending.append((csh[1], base + 512, "D"))
                flush_pending()
                nc.sync.dma_start(den_d[:], den_acc[:])
                nc.sync.dma_start(orow_d[:], orow[:])

    nc.compile()
    return nc


_NC = None


def _get_nc():
    global _NC
    if _NC is None:
        _NC = build_nc()
    return _NC


def make_in_maps(x1, x2):
    x1 = np.asarray(x1, dtype=np.float32)
    x2 = np.asarray(x2, dtype=np.float32)
    x = np.concatenate([x1, x2], axis=0)               # [8192, 256]
    xT8 = np.ascontiguousarray(x.T).astype(ml_dtypes.float8_e4m3fn)
    # reciprocal norms of the fp8-quantized columns, bf16 (device semantics)
    ssq = (xT8.astype(np.float32) ** 2).sum(axis=0)    # [8192]
    r_full = (SCALE_Z / np.sqrt(ssq)).astype(ml_dtypes.bfloat16)
    in_maps = []
    for c in range(8):
        xr = np.roll(xT8, -c * RPC, axis=1)[:, :SW]
        arr = np.stack([xr[:128], xr[128:]], axis=1)   # [128, 2, SW]
        rr = np.roll(r_full, -c * RPC)[:SW]
        in_maps.append({
            "xt": np.ascontiguousarray(arr.reshape(128, 2 * SW)),
            "r": np.ascontiguousarray(
                np.broadcast_to(rr[None, :], (128, SW))),
        })
    return in_maps


def _combine(results):
    """Host-side unshard: assemble denominators, diag correction, loss."""
    den_total = np.zeros(TWO_N, dtype=np.float64)
    pos_sum = 0.0
    for c in range(8):
        r = results[c]
        den_own = np.asarray(r["den"], dtype=np.float64)   # [128, 40]
        orow = np.asarray(r["orow"], dtype=np.float64).reshape(6 * RPC)
        # rowsums over all 5 panels (g0 covers cols >= its diag chunk);
        # local row i = m*128 + p
        den_rows = den_own.reshape(128, M_TILES, G).sum(axis=2)
        den_rows = den_rows.T.reshape(RPC)                  # [i]
        slf = orow[4 * RPC:5 * RPC]
        den_rows = den_rows + 1.0 - np.exp(K_SIM * slf)
        lo = c * RPC
        den_total[lo:lo + RPC] += den_rows
        cs = orow[:3 * RPC].reshape(3, RPC)
        for g in (1, 2, 3):
            dest = ((c + g) * RPC) % TWO_N
            den_total[dest:dest + RPC] += cs[g - 1]
        pos_sum += float(orow[3 * RPC:4 * RPC].sum())
    loss = (np.log(den_total).sum() - K_SIM * pos_sum) / TWO_N
    return np.asarray(np.float32(loss))


def _run(x1, x2, trace=False, tmpdir=None):
    from concourse.bass_utils import run_bass_kernel_spmd

    nc = _get_nc()
    in_maps = make_in_maps(x1, x2)
    res = run_bass_kernel_spmd(
        nc, in_maps, list(range(8)), trace=trace, tmpdir=tmpdir
    )
    loss = _combine(res.results)
    return loss, res


def kernel(x1, x2):
    loss, _ = _run(x1, x2)
    return loss
